# revision 4
# baseline (speedup 1.0000x reference)
"""Trainium2 Bass kernel for a 15-layer LSTM encoder + 20-step autoregressive
decoder regressor (H=64, T=20, B=8192) on 8 NeuronCores, pure data parallel.

Layout (per core, batch shard BS=1024 split into two halves of HB=512):
  - Per layer l: combined rhs tiles A_l/B_l [128, 512] bf16:
      partitions 0:64  = layer input (h of layer l-1; layer 0: row0=x_t, row1=ones)
      partitions 64:128 = recurrent h of layer l
    One K=128 matmul per (gate, half) computes Wih@x + Whh@h (+bias for l=0).
  - Gate PSUM banks [g_h1; g_h2] [128, 512] fp32, one per gate (i,f,o,g),
    written by a col-tiled matmul pair (M=64 at cols 0:64 / 64:128).
  - ACT applies sigmoid/tanh with fused per-partition bias, PSUM->SBUF bf16.
  - c state [128, 512] fp32, updated in place on DVE.
  - Encode runs as a (layer, time) wavefront; decode is the serial
    layer chain with the fc feedback, h propagated by DVE, preds DMA'd out.
"""

import numpy as np
import ml_dtypes

import concourse.bass as bass
import concourse.bacc as bacc
import concourse.tile as tile
import concourse.mybir as mybir
from concourse.bass_utils import run_bass_kernel_spmd

BF16 = mybir.dt.bfloat16
F32 = mybir.dt.float32
AF = mybir.ActivationFunctionType

H = 64
NCORES = 8
HB = 512          # half-batch per core
BS = 2 * HB       # batch shard per core

# gate processing order (emission): g first (tanh feeds t1), then i, f, o
# torch gate row-block order in the 4H dim is [i, f, g, o]
GATE_ORDER = [("g", 2, AF.Tanh), ("i", 0, AF.Sigmoid), ("f", 1, AF.Sigmoid), ("o", 3, AF.Sigmoid)]


def build_nc(L, T):
    """Builds the Bass program. Returns nc."""
    nc = bacc.Bacc("TRN2", target_bir_lowering=False, debug=False)

    xprep = nc.dram_tensor("xprep", (T + 1, 4 * HB), BF16, kind="ExternalInput")
    w_all = nc.dram_tensor("w_all", (128, L * 4 * 64), BF16, kind="ExternalInput")
    w0d = nc.dram_tensor("w0d", (128, 4 * 64), BF16, kind="ExternalInput")
    bias_all = nc.dram_tensor("bias_all", (128, L * 4), F32, kind="ExternalInput")
    fc_wT = nc.dram_tensor("fc_wT", (128, 1), BF16, kind="ExternalInput")
    preds = nc.dram_tensor("preds", (BS, T), BF16, kind="ExternalOutput")

    from contextlib import ExitStack
    with tile.TileContext(nc) as tc, ExitStack() as ctx:
        const = ctx.enter_context(tc.tile_pool(name="const", bufs=1))
        work = ctx.enter_context(tc.tile_pool(name="work", bufs=3))
        psum = ctx.enter_context(tc.tile_pool(name="psum", bufs=8, space="PSUM"))

        # ---- load constants ----
        w_sb = const.tile([128, L * 4 * 64], BF16, tag="w")
        nc.sync.dma_start(out=w_sb[:], in_=w_all[:])
        w0d_sb = const.tile([128, 4 * 64], BF16, tag="w0d")
        nc.sync.dma_start(out=w0d_sb[:], in_=w0d[:])
        bias_sb = const.tile([128, L * 4], F32, tag="bias")
        nc.sync.dma_start(out=bias_sb[:], in_=bias_all[:])
        fc_sb = const.tile([128, 1], BF16, tag="fc")
        nc.sync.dma_start(out=fc_sb[:], in_=fc_wT[:])

        # ---- state tiles ----
        A = [const.tile([128, HB], BF16, tag=f"A{l}", name=f"A{l}") for l in range(L)]
        B = [const.tile([128, HB], BF16, tag=f"B{l}", name=f"B{l}") for l in range(L)]
        C = [const.tile([128, HB], F32, tag=f"c{l}", name=f"C{l}") for l in range(L)]
        for l in range(L):
            nc.gpsimd.memset(A[l][:], 0.0)
            nc.gpsimd.memset(B[l][:], 0.0)
            nc.gpsimd.memset(C[l][:], 0.0)

        def wslice(l, gi, decode):
            if l == 0 and decode:
                return w0d_sb[:, gi * 64:(gi + 1) * 64]
            return w_sb[:, (l * 4 + gi) * 64:(l * 4 + gi) * 64 + 64]

        def emit_cell(l, decode, last_layer_sink=None):
            """One LSTM cell update for layer l on the current A/B/C state.

            last_layer_sink: for decode l==L-1, a callable run with (So, Tc)
            available to also produce the fc output.
            """
            banks = {}
            for gname, gi, _fn in GATE_ORDER:
                bank = psum.tile([128, HB], F32, tag="gates")
                lw = wslice(l, gi, decode)
                nc.tensor.matmul(bank[0:64, :], lhsT=lw, rhs=A[l][:], start=True, stop=True)
                nc.tensor.matmul(bank[64:128, :], lhsT=lw, rhs=B[l][:], start=True, stop=True)
                banks[gname] = bank

            S = {}
            for gname, gi, fn in GATE_ORDER:
                sg = work.tile([128, HB], BF16, tag=f"S{gname}")
                bias_ap = 0.0 if l == 0 else bias_sb[:, l * 4 + gi:l * 4 + gi + 1]
                nc.scalar.activation(out=sg[:], in_=banks[gname][:], func=fn, bias=bias_ap)
                S[gname] = sg

            t1 = work.tile([128, HB], BF16, tag="t1")
            nc.vector.tensor_mul(out=t1[:], in0=S["i"][:], in1=S["g"][:])
            nc.vector.tensor_mul(out=C[l][:], in0=S["f"][:], in1=C[l][:])
            nc.vector.tensor_add(out=C[l][:], in0=C[l][:], in1=t1[:])
            tc_t = work.tile([128, HB], BF16, tag="Tc")
            nc.scalar.activation(out=tc_t[:], in_=C[l][:], func=AF.Tanh)

            # h update into own recurrent slot
            nc.vector.tensor_mul(out=A[l][64:128, :], in0=S["o"][0:64, :], in1=tc_t[0:64, :])
            nc.vector.tensor_mul(out=B[l][64:128, :], in0=S["o"][64:128, :], in1=tc_t[64:128, :])

            if l < L - 1:
                if decode:
                    # chain-critical: recompute h straight into next layer's input slot
                    nc.vector.tensor_mul(out=A[l + 1][0:64, :], in0=S["o"][0:64, :], in1=tc_t[0:64, :])
                    nc.vector.tensor_mul(out=B[l + 1][0:64, :], in0=S["o"][64:128, :], in1=tc_t[64:128, :])
                else:
                    nc.sync.dma_start(out=A[l + 1][0:64, :], in_=A[l][64:128, :])
                    nc.sync.dma_start(out=B[l + 1][0:64, :], in_=B[l][64:128, :])

        # ================= encode: wavefront over (l, t) =================
        for s in range(L + T - 1):
            for l in range(max(0, s - (T - 1)), min(L, s + 1)):
                t = s - l
                if l == 0:
                    nc.sync.dma_start(out=A[0][0:2, :], in_=xprep[t:t + 1, 0:2 * HB])
                    nc.sync.dma_start(out=B[0][0:2, :], in_=xprep[t:t + 1, 2 * HB:4 * HB])
                emit_cell(l, decode=False)

        # ================= decode: serial chain =================
        for t in range(T):
            # layer-0 input
            if t == 0:
                nc.sync.dma_start(out=A[0][0:2, :], in_=xprep[T:T + 1, 0:2 * HB])
                nc.sync.dma_start(out=B[0][0:2, :], in_=xprep[T:T + 1, 2 * HB:4 * HB])
            for l in range(L):
                emit_cell(l, decode=True)
            # fc head: y_tilde = fc_w @ h_last  (fc_b folded in host-side)
            fcA = psum.tile([1, HB], F32, tag="gates")
            fcB = psum.tile([1, HB], F32, tag="gates")
            nc.tensor.matmul(fcA[0:1, :], lhsT=fc_sb[64:128, 0:1], rhs=A[L - 1][64:128, :],
                             start=True, stop=True)
            nc.tensor.matmul(fcB[0:1, :], lhsT=fc_sb[64:128, 0:1], rhs=B[L - 1][64:128, :],
                             start=True, stop=True)
            # feed next step's layer-0 input (bf16 cast) and stage preds
            nc.vector.tensor_copy(out=A[0][0:1, :], in_=fcA[0:1, :])
            nc.vector.tensor_copy(out=B[0][0:1, :], in_=fcB[0:1, :])
            nc.sync.dma_start(out=preds[0:HB, t:t + 1], in_=A[0][0:1, :])
            nc.sync.dma_start(out=preds[HB:2 * HB, t:t + 1], in_=B[0][0:1, :])

    nc.compile()
    return nc


def prep_inputs(x, W_ih0, W_hh0, b_ih0, b_hh0, W_ih, W_hh, b_ih, b_hh, fc_w, fc_b, L, T):
    """Host-side packing into the machine layouts. Returns (common, per_core_xprep list)."""
    fcb = float(np.asarray(fc_b).reshape(-1)[0])
    b0 = np.asarray(b_ih0, np.float32) + np.asarray(b_hh0, np.float32)  # [4H]

    def gate_block(M, gi):
        return np.asarray(M, np.float32)[gi * H:(gi + 1) * H]

    # w_all [128, L*4*64]
    w_all = np.zeros((128, L * 4 * 64), np.float32)
    w0d = np.zeros((128, 4 * 64), np.float32)
    bias_all = np.zeros((128, L * 4), np.float32)
    # order in w_all columns is by gi = torch block index used in wslice
    for l in range(L):
        for gi in range(4):
            col = (l * 4 + gi) * 64
            if l == 0:
                wih_g = gate_block(W_ih0, gi)          # [64, 1]
                whh_g = gate_block(W_hh0, gi)          # [64, 64]
                blk = np.zeros((128, 64), np.float32)
                blk[0, :] = wih_g[:, 0]
                blk[1, :] = b0[gi * H:(gi + 1) * H]
                blk[64:128, :] = whh_g.T
                w_all[:, col:col + 64] = blk
                blk_d = blk.copy()
                blk_d[1, :] = b0[gi * H:(gi + 1) * H] + wih_g[:, 0] * fcb
                w0d[:, gi * 64:gi * 64 + 64] = blk_d
            else:
                wih_g = gate_block(W_ih[l - 1], gi)    # [64, 64]
                whh_g = gate_block(W_hh[l - 1], gi)
                blk = np.zeros((128, 64), np.float32)
                blk[0:64, :] = wih_g.T
                blk[64:128, :] = whh_g.T
                w_all[:, col:col + 64] = blk
                bsum = (np.asarray(b_ih[l - 1], np.float32) + np.asarray(b_hh[l - 1], np.float32))
                bg = bsum[gi * H:(gi + 1) * H]
                bias_all[0:64, l * 4 + gi] = bg
                bias_all[64:128, l * 4 + gi] = bg

    fc_wT = np.zeros((128, 1), np.float32)
    fc_wT[64:128, 0] = np.asarray(fc_w, np.float32)[0, :]

    common = {
        "w_all": w_all.astype(ml_dtypes.bfloat16),
        "w0d": w0d.astype(ml_dtypes.bfloat16),
        "bias_all": bias_all,
        "fc_wT": fc_wT.astype(ml_dtypes.bfloat16),
    }

    # per-core xprep
    x = np.asarray(x, np.float32).reshape(x.shape[0], x.shape[1])  # [B, T]
    Bfull = x.shape[0]
    bs = Bfull // NCORES
    xpreps = []
    for c in range(NCORES):
        xs = x[c * bs:(c + 1) * bs]  # [BS, T]
        xp = np.zeros((T + 1, 4 * HB), np.float32)
        for t in range(T):
            xp[t, 0:HB] = xs[0:HB, t]
            xp[t, 2 * HB:3 * HB] = xs[HB:2 * HB, t]
        xp[T, 0:HB] = xs[0:HB, T - 1] - fcb
        xp[T, 2 * HB:3 * HB] = xs[HB:2 * HB, T - 1] - fcb
        xp[:, HB:2 * HB] = 1.0
        xp[:, 3 * HB:4 * HB] = 1.0
        xpreps.append(xp.astype(ml_dtypes.bfloat16))
    return common, xpreps, fcb


_NC_CACHE = {}


def run(inputs, L, T, trace=False):
    key = (L, T)
    if key not in _NC_CACHE:
        _NC_CACHE[key] = build_nc(L, T)
    nc = _NC_CACHE[key]
    common, xpreps, fcb = prep_inputs(L=L, T=T, **inputs)
    in_maps = [dict(common, xprep=xp) for xp in xpreps]
    res = run_bass_kernel_spmd(nc, in_maps, core_ids=list(range(NCORES)), trace=trace)
    parts = [r["preds"].astype(np.float32) + fcb for r in res.results]
    return np.concatenate(parts, axis=0), res


def kernel(**inputs):
    out, _ = run(inputs, L=15, T=20)
    return out
